# revision 9
# baseline (speedup 1.0000x reference)
"""BCJR detector kernel v2 for Trainium2: fused radix-3 alpha/beta chains.

Math (validated in proto.py):
  c-space alpha (8 states):  c_{t+3} = M3_t @ c_t            (forward)
  e-space beta  (8 states):  e_t     = N3_t @ e_{t+3}        (backward)
  M3[j,i] = g0[(j>>2)+2b0+4b1+8b2] * g1[(j>>1)+4b0+8b1] * g2[j+8b0]
  N3[j,i] = g0[2j+w0] * g1[(4j+2w0)%16+w1] * g2[(8j+4w0+2w1)%16+w2]
     b0=i&1, b1=(i>>1)&1, b2=(i>>2)&1 ; w0=(i>>2)&1, w1=(i>>1)&1, w2=i&1
  b_t[s]  = e_{t+1}[s%8] * g_t[s]
  F = g_t * b_t ;  upd_t = sum_j c_t[j]*(F[2j]-F[2j+1]) ; decoded = upd<0

Per core: 128 words on partitions, time on the free dim.  The two chains
advance together: fused step = one tensor_tensor mult [p,2x8x8] (bf16,
DVE 2x mode) + one tensor_reduce, alpha group n and beta group NG-1-n.

g layout: t-major, state cols permuted PIG = s1@1, s0@2, s2@4, s3@8 so the
radix-3 weight builds hit the DVE 2x packed-innermost condition.
"""

import math
import sys

import numpy as np

sys.path.insert(0, "/opt/trn_rl_repo")

B, T, S, MEM, V = 1024, 2048, 16, 4, 4
NCORES = 8
BPC = B // NCORES

NB = 11          # fused block pairs
GPB = 62         # radix-3 groups per block
NORM_G = 128     # normalize every NORM_G groups

PIG = [0] * 16
for _s in range(16):
    PIG[_s] = (((_s >> 1) & 1) + 2 * (_s & 1) + 4 * ((_s >> 2) & 1)
               + 8 * ((_s >> 3) & 1))


def _mkap(win, dims):
    """Custom strided view: win is a [:, a:b] AP window giving base offset."""
    import bass_rust
    v = win.copy()
    part = list(v.ap)[0]
    v.ap = bass_rust.VecI64Pair([list(part)] + [list(d) for d in dims])
    return v


def _build(nc, cfg):
    import concourse.bass as bass  # noqa: F401
    from concourse import mybir, tile
    from concourse.alu_op_type import AluOpType as OP
    from concourse.mybir import ActivationFunctionType as AF

    f32 = mybir.dt.float32
    bf16 = mybir.dt.bfloat16
    nb, gpb = cfg["NB"], cfg["GPB"]
    ng = nb * gpb
    tc = 3 * ng
    bt = 3 * gpb
    Tn = tc + 2
    scale, bias = float(cfg["scale"]), float(cfg["bias"])
    norm_g = cfg["NORM_G"]

    yin_d = nc.dram_tensor("yin", [BPC, Tn + S], f32, kind="ExternalInput")
    out_d = nc.dram_tensor("dec", [BPC, tc], f32, kind="ExternalOutput")

    ctx = nc.allow_low_precision(reason="bf16 chain state; decisions are sign-based")
    ctx.__enter__()
    with tile.TileContext(nc) as tc_:
        with (
            tc_.tile_pool(name="big", bufs=1) as big,
            tc_.tile_pool(name="dring", bufs=1) as dring,
            tc_.tile_pool(name="m3ring", bufs=2) as m3ring,
            tc_.tile_pool(name="p2ring", bufs=1) as p2ring,
            tc_.tile_pool(name="cring", bufs=1) as cring,
            tc_.tile_pool(name="sm", bufs=1) as sm,
        ):
            spt = big.tile([BPC, S], f32, tag="spt")
            sp_sb = spt[:, :]                    # PIG-ordered sp per word
            gfull = big.tile([BPC, 16 * Tn], bf16, tag="gfull")
            stslab = big.tile([BPC, 16 * (ng + 1)], bf16, tag="stslab")
            cfill_b = [big.tile([BPC, 16 * gpb], bf16, name=f"cfill{b_}",
                                tag=f"cfill{b_}") for b_ in range(nb)]
            efill_b = [big.tile([BPC, 16 * gpb], bf16, name=f"efill{b_}",
                                tag=f"efill{b_}") for b_ in range(nb)]
            rsc = sm.tile([BPC, 1], f32, tag="rsc")
            ssum = sm.tile([BPC, 1], f32, tag="ssum")
            bias_t = sm.tile([BPC, 1], f32, tag="bias")
            zero_t = sm.tile([BPC, 1], f32, tag="zero")
            e1t = sm.tile([BPC, 8], bf16, tag="e1")
            btmp = sm.tile([BPC, 16], bf16, tag="btmp")
            ctmp = sm.tile([BPC, 128], bf16, tag="ctmp")
            nc.vector.memset(bias_t[:, :], bias)
            nc.vector.memset(zero_t[:, :], 0.0)

            nc.sync.dma_start(spt[:, :], yin_d[:, Tn:Tn + S])

            def gen_g(blk_t0, n, which, eng=None):
                """g[16*t + PIG[s]] = exp(scale*(y[t]-sp[s])^2 + bias)."""
                d = dring.tile([BPC, 16 * n], bf16, name=f"d_{which}", tag=f"d{n}")
                ystrip = dring.tile([BPC, n], f32, name=f"y_{which}", tag=f"y{n}")
                nc.sync.dma_start(ystrip[:, :], yin_d[:, blk_t0:blk_t0 + n])
                dv = d[:, :].rearrange("p (t s) -> p t s", s=16)
                yv = ystrip[:, :].unsqueeze(2).broadcast_to((BPC, n, 16))
                spv = sp_sb[:, :].unsqueeze(1).broadcast_to((BPC, n, 16))
                (eng or nc.gpsimd).tensor_tensor(dv, yv, spv, OP.subtract)
                nc.scalar.activation(d[:, :], d[:, :], AF.Square,
                                     bias=zero_t[:, :], scale=1.0)
                gv = gfull[:, 16 * blk_t0:16 * (blk_t0 + n)]
                nc.scalar.activation(gv, d[:, :], AF.Exp,
                                     bias=bias_t[:, :], scale=scale)

            def beta_init():
                """slot 0 beta half = e_{tc}; needs g at t = tc, tc+1."""
                gen_g(tc, 2, "binit")
                nc.vector.memset(e1t[:, :], 0.0)
                c0 = 16 * (tc + 1)
                nc.vector.tensor_copy(e1t[:, 0:1], gfull[:, c0:c0 + 1])
                nc.vector.tensor_copy(e1t[:, 4:5], gfull[:, c0 + 8:c0 + 9])
                # btmp[8j2+4j1+2j0+u] = e1[(2j+u)%8] * g_tc[2j+u]
                # iterate (j2, j1, j0, u):
                #   e col 4j1+2j0+u ; g col 16*tc + pig(j2,j1,j0,u)=(8,4,1,2)
                for j2 in range(2):
                    ev = _mkap(e1t[:, 0:1], [(4, 2), (2, 2), (1, 2)])
                    gv = _mkap(gfull[:, 16 * tc + 8 * j2:16 * tc + 8 * j2 + 1],
                               [(4, 2), (1, 2), (2, 2)])
                    tv = _mkap(btmp[:, 8 * j2:8 * j2 + 1],
                               [(4, 2), (2, 2), (1, 2)])
                    nc.vector.tensor_tensor(tv, ev, gv, OP.mult)
                nc.vector.tensor_tensor(stslab[:, 8:16], btmp[:, 0:16:2],
                                        btmp[:, 1:16:2], OP.add)

            def build_m3(ablk, bblk, tau, m3, p2a, p2b, k0, klen):
                """Fused weights for fused slots [k0, k0+klen).

                All instructions split so every AP is <= 4 free dims
                (walrus limit), preserving packed-innermost 2x on DVE.
                """
                t0 = 3 * (ablk * gpb + k0)
                # P2a[32k+4j+2b1+b0] = g1[(j>>1)+4b0+8b1] * g2[j+8b0]
                # split per (b1, b0); iterate (k, j2, j1, j0)
                eng_p2 = nc.gpsimd
                for b1 in range(2):
                    for b0 in range(2):
                        for j2 in range(2):
                            g1v = _mkap(
                                gfull[:, 16 * (t0 + 1) + 8 * b1 + 4 * b0 + j2:
                                      16 * (t0 + 1) + 8 * b1 + 4 * b0 + j2 + 1],
                                [(48, klen), (2, 2), (0, 2)])
                            g2v = _mkap(
                                gfull[:, 16 * (t0 + 2) + 8 * b0 + 4 * j2:
                                      16 * (t0 + 2) + 8 * b0 + 4 * j2 + 1],
                                [(48, klen), (1, 2), (2, 2)])
                            ob = 32 * k0 + 16 * j2 + 2 * b1 + b0
                            p2av = _mkap(p2a[:, ob:ob + 1],
                                         [(32, klen), (8, 2), (4, 2)])
                            eng_p2.tensor_tensor(p2av, g1v, g2v, OP.mult)
                # M3[128k+8j+i] = P2a[k,j,b1,b0] * g0[(j>>2)+2b0+4b1+8b2]
                # split per j; iterate (k, b2, b1, b0); i = 4b2+2b1+b0
                eng_af = (nc.vector if (tau >= nb // 2 or tau == 0)
                          else nc.gpsimd)
                for j in range(8):
                    for b2 in range(2):
                        gb = 16 * t0 + 2 * (j >> 2) + 8 * b2
                        g0v = _mkap(gfull[:, gb:gb + 1],
                                    [(48, klen), (4, 2), (1, 2)])
                        p2av2 = _mkap(
                            p2a[:, 32 * k0 + 4 * j:32 * k0 + 4 * j + 1],
                            [(32, klen), (2, 2), (1, 2)])
                        mb = 128 * k0 + 8 * j + 4 * b2
                        m3av = _mkap(m3[:, mb:mb + 1],
                                     [(128, klen), (2, 2), (1, 2)])
                        eng_af.tensor_tensor(m3av, p2av2, g0v, OP.mult)

                # beta: fused slot k holds beta group m = bblk*gpb + gpb-1-k
                tb = 3 * (bblk * gpb + gpb - 1 - k0)  # beta t0 at krel=k0
                # P2b[32k+4j+2w0+w1] = g0[2j+w0] * g1[(4j+2w0)%16+w1]
                # split per (w0, w1); iterate (k, j2, j1, j0)
                # g0 s-bits (j2,j1,j0,w0); g1 s-bits (j1,j0,w0,w1)
                for w0 in range(2):
                    for w1 in range(2):
                        for j2 in range(2):
                            gb0 = 16 * tb + 2 * w0 + 8 * j2
                            g0b = _mkap(gfull[:, gb0:gb0 + 1],
                                        [(-48, klen), (4, 2), (1, 2)])
                            gb1 = 16 * (tb + 1) + w0 + 2 * w1
                            g1b = _mkap(gfull[:, gb1:gb1 + 1],
                                        [(-48, klen), (8, 2), (4, 2)])
                            ob = 32 * k0 + 16 * j2 + 2 * w0 + w1
                            p2bv = _mkap(p2b[:, ob:ob + 1],
                                         [(32, klen), (8, 2), (4, 2)])
                            eng_p2.tensor_tensor(p2bv, g0b, g1b, OP.mult)
                # N3[128k+64+8j+i] = P2b * g2[(8j+4w0+2w1)%16+w2], i=4w0+2w1+w2
                # split per j; iterate (k, w0, w1, w2)
                # g2 s-bits (j0, w0, w1, w2) -> pig j0@8, w0@4, w1@1, w2@2
                eng_bf = nc.vector if tau == 0 else nc.gpsimd
                for j in range(8):
                    for w0 in range(2):
                        gb = 16 * (tb + 2) + 8 * (j & 1) + 4 * w0
                        g2b = _mkap(gfull[:, gb:gb + 1],
                                    [(-48, klen), (1, 2), (2, 2)])
                        pb = 32 * k0 + 4 * j + 2 * w0
                        p2bv2 = _mkap(p2b[:, pb:pb + 1],
                                      [(32, klen), (1, 2), (0, 2)])
                        mb = 128 * k0 + 64 + 8 * j + 4 * w0
                        m3bv = _mkap(m3[:, mb:mb + 1],
                                     [(128, klen), (2, 2), (1, 2)])
                        eng_bf.tensor_tensor(m3bv, p2bv2, g2b, OP.mult)

            def chain_block(tau, m3, k0, klen):
                for krel in range(k0, k0 + klen):
                    n = tau * gpb + krel
                    slot = stslab[:, 16 * n:16 * n + 16]
                    nxt = stslab[:, 16 * n + 16:16 * n + 32]
                    in0 = (slot.rearrange("p (seg i) -> p seg i", i=8)
                           .unsqueeze(2).broadcast_to((BPC, 2, 8, 8)))
                    m3v = m3[:, 128 * krel:128 * (krel + 1)].rearrange(
                        "p (seg j i) -> p seg j i", seg=2, i=8)
                    tv = ctmp[:, :].rearrange("p (seg j i) -> p seg j i",
                                              seg=2, i=8)
                    nc.vector.tensor_tensor(tv, in0, m3v, OP.mult)
                    t3 = ctmp[:, :].rearrange("p (sj i) -> p sj i", i=8)
                    nc.vector.tensor_reduce(nxt, t3, mybir.AxisListType.X,
                                            OP.add)
                    if (n + 1) % norm_g == 0:
                        nc.vector.tensor_reduce(ssum[:, :], nxt,
                                                mybir.AxisListType.X, OP.add)
                        nc.vector.reciprocal(rsc[:, :], ssum[:, :])
                        nc.vector.tensor_scalar(nxt, nxt, rsc[:, :], None,
                                                OP.mult)

            def fills_alpha(ablk, fa):
                t0 = 3 * ablk * gpb
                k0 = ablk * gpb
                for r in (1, 2):
                    # tmp layout: col 32k + 16u + 4j2 + 2j0 + j1
                    # split per u; iterate (k, j2, j0, j1):
                    #   g s-bits (u, j2, j1, j0): pig -> u@8, j2@4, j1@1, j0@2
                    for u in range(2):
                        for j2 in range(2):
                            gb = 16 * (t0 + r - 1) + 8 * u + 4 * j2
                            gv = _mkap(gfull[:, gb:gb + 1],
                                       [(48, gpb), (2, 2), (1, 2)])
                            cb = 4 * u + 2 * j2
                            if r == 1:
                                cv = _mkap(stslab[:, 16 * k0 + cb:
                                                  16 * k0 + cb + 1],
                                           [(16, gpb), (0, 2), (1, 2)])
                            else:
                                cv = _mkap(cfill_b[ablk][:, cb:cb + 1],
                                           [(16, gpb), (0, 2), (1, 2)])
                            tb_ = 16 * u + 4 * j2
                            tv = _mkap(fa[:, tb_:tb_ + 1],
                                       [(32, gpb), (2, 2), (1, 2)])
                            nc.gpsimd.tensor_tensor(tv, cv, gv, OP.mult)
                    # cfill[16k+8(r-1)+j] = tmp[k,0,j] + tmp[k,1,j]
                    # iterate (k, j2, j1, j0): tmp (32, 4, 1, 2)
                    for j2 in range(2):
                        ob = 8 * (r - 1) + 4 * j2
                        ov = _mkap(cfill_b[ablk][:, ob:ob + 1],
                                   [(16, gpb), (2, 2), (1, 2)])
                        t0v = _mkap(fa[:, 4 * j2:4 * j2 + 1],
                                    [(32, gpb), (1, 2), (2, 2)])
                        t1v = _mkap(fa[:, 16 + 4 * j2:16 + 4 * j2 + 1],
                                    [(32, gpb), (1, 2), (2, 2)])
                        nc.gpsimd.tensor_tensor(ov, t0v, t1v, OP.add)

            def fills_beta(bblk, fb):
                # efill layout sesB: value m at col 16k + 8*(r-1) + 4m2+m1+2m0
                # fb tmp layout: b value s at col 32k + PIG(s)
                t0 = 3 * bblk * gpb
                k0 = bblk * gpb
                for r in (2, 1):
                    # b_t[2j+u] = e_{t+1}[(2j+u)%8]*g_t[2j+u], t = 3k+r
                    # split per u; iterate (k, j2, j1, j0)
                    #   g s-bits (j2, j1, j0, u): pig -> j2@8, j1@4, j0@1, u@2
                    for u in range(2):
                        for j2 in range(2):
                            gb = 16 * (t0 + r) + 2 * u + 8 * j2
                            gv = _mkap(gfull[:, gb:gb + 1],
                                       [(48, gpb), (4, 2), (1, 2)])
                            # e value (2j+u)%8: bits (m2,m1,m0)=(j1,j0,u)
                            if r == 2:
                                basecol = 16 * (ng - (k0 + 1)) + 8 + u
                                ev = _mkap(stslab[:, basecol:basecol + 1],
                                           [(-16, gpb), (4, 2), (2, 2)])
                            else:
                                eb = 8 + 2 * u
                                ev = _mkap(efill_b[bblk][:, eb:eb + 1],
                                           [(16, gpb), (4, 2), (1, 2)])
                            tb_ = 2 * u + 8 * j2
                            tv = _mkap(fb[:, tb_:tb_ + 1],
                                       [(32, gpb), (4, 2), (1, 2)])
                            nc.gpsimd.tensor_tensor(tv, ev, gv, OP.mult)
                    # pairadd: efill sesB out; iterate (k, j2, j0, j1):
                    #   out col 4j2 + j1 + 2j0 -> (16, 4, 2, 1)
                    #   tmp s=2j (PIG: j2@8, j1@4, j0@1): (32, 8, 1, 4); odd +2
                    for j2 in range(2):
                        ob = 8 * (r - 1) + 4 * j2
                        ov = _mkap(efill_b[bblk][:, ob:ob + 1],
                                   [(16, gpb), (2, 2), (1, 2)])
                        t0v = _mkap(fb[:, 8 * j2:8 * j2 + 1],
                                    [(32, gpb), (1, 2), (4, 2)])
                        t1v = _mkap(fb[:, 8 * j2 + 2:8 * j2 + 3],
                                    [(32, gpb), (1, 2), (4, 2)])
                        nc.gpsimd.tensor_tensor(ov, t0v, t1v, OP.add)

            def build_b(cblk, bf, engp=None):
                engp = engp or nc.gpsimd
                """b_t[s] = e_{t+1}[s%8]*g_t[s]; bf col = 16*trel + PIG(s)."""
                t0 = 3 * cblk * gpb
                k0 = cblk * gpb
                for r in range(3):
                    # split per (s3, s2); iterate (k, s0, s1)
                    for s3 in range(2):
                        for s2 in range(2):
                            gb = 16 * (t0 + r) + 8 * s3 + 4 * s2
                            gv = _mkap(gfull[:, gb:gb + 1],
                                       [(48, gpb), (2, 2), (1, 2)])
                            if r == 0:
                                ev = _mkap(efill_b[cblk][:, 4 * s2:4 * s2 + 1],
                                           [(16, gpb), (2, 2), (1, 2)])
                            elif r == 1:
                                eb = 8 + 4 * s2
                                ev = _mkap(efill_b[cblk][:, eb:eb + 1],
                                           [(16, gpb), (2, 2), (1, 2)])
                            else:
                                basecol = 16 * (ng - (k0 + 1)) + 8 + 4 * s2
                                ev = _mkap(stslab[:, basecol:basecol + 1],
                                           [(-16, gpb), (1, 2), (2, 2)])
                            bb = 16 * r + 8 * s3 + 4 * s2
                            bv = _mkap(bf[:, bb:bb + 1],
                                       [(48, gpb), (2, 2), (1, 2)])
                            engp.tensor_tensor(bv, ev, gv, OP.mult)

            def combine(cblk, bf, Ft, Dt, cDt, updt, dect):
                t0 = 3 * cblk * gpb
                k0 = cblk * gpb
                # F = g*b, both PIG t-major: plain contiguous mult (2x)
                engp.tensor_tensor(Ft[:, :],
                                   gfull[:, 16 * t0:16 * (t0 + bt)],
                                   bf[:, :], OP.mult)
                # D[8trel + j] = F[s=2j] - F[s=2j+1]; PIG: s0@2, j=(s3,s2,s1)
                # iterate (trel, j2, j1, j0): F (16, 8, 4, 1); odd +2
                for j2 in range(2):
                    fe = _mkap(Ft[:, 8 * j2:8 * j2 + 1],
                               [(16, bt), (4, 2), (1, 2)])
                    fo = _mkap(Ft[:, 8 * j2 + 2:8 * j2 + 3],
                               [(16, bt), (4, 2), (1, 2)])
                    dv = _mkap(Dt[:, 4 * j2:4 * j2 + 1],
                               [(8, bt), (2, 2), (1, 2)])
                    nc.vector.tensor_tensor(dv, fe, fo, OP.subtract)
                # cD[8trel+j] = c_t[j]*D; c src: r=0 stslab slot, r>0 cfill
                for r in range(3):
                    if r == 0:
                        cv = _mkap(stslab[:, 16 * k0:16 * k0 + 1],
                                   [(16, gpb), (4, 2), (2, 2), (1, 2)])
                    else:
                        cv = _mkap(cfill_b[C][:, 8 * (r - 1):8 * (r - 1) + 1],
                                   [(16, gpb), (4, 2), (2, 2), (1, 2)])
                    drv = _mkap(Dt[:, 8 * r:8 * r + 1],
                                [(24, gpb), (4, 2), (2, 2), (1, 2)])
                    cdv = _mkap(cDt[:, 8 * r:8 * r + 1],
                                [(24, gpb), (4, 2), (2, 2), (1, 2)])
                    nc.vector.tensor_tensor(cdv, cv, drv, OP.mult)
                cd3 = cDt[:, :].rearrange("p (t j) -> p t j", j=8)
                nc.vector.tensor_reduce(updt[:, :], cd3, mybir.AxisListType.X,
                                        OP.add)
                nc.gpsimd.tensor_scalar(dect[:, :], updt[:, :], 0.0, None,
                                        OP.is_lt)
                nc.sync.dma_start(out_d[:, t0:t0 + bt], dect[:, :])

            # ================= main schedule ===============================
            nc.vector.memset(stslab[:, 0:8], 0.0)
            nc.vector.memset(stslab[:, 0:1], 1.0)

            def combine_pool(C, engp=None):
                engp = engp or nc.gpsimd
                bf = cring.tile([BPC, 16 * bt], bf16, name=f"bf{C}", tag="bf")
                build_b(C, bf, engp)
                Ft = cring.tile([BPC, 16 * bt], bf16, name=f"F{C}", tag="F")
                Dt = cring.tile([BPC, 8 * bt], bf16, name=f"D{C}", tag=f"D{C % 2}")
                t0 = 3 * C * gpb
                engp.tensor_tensor(Ft[:, :],
                                   gfull[:, 16 * t0:16 * (t0 + bt)],
                                   bf[:, :], OP.mult)
                for j2 in range(2):
                    fe = _mkap(Ft[:, 8 * j2:8 * j2 + 1],
                               [(16, bt), (4, 2), (1, 2)])
                    fo = _mkap(Ft[:, 8 * j2 + 2:8 * j2 + 3],
                               [(16, bt), (4, 2), (1, 2)])
                    dv = _mkap(Dt[:, 4 * j2:4 * j2 + 1],
                               [(8, bt), (2, 2), (1, 2)])
                    nc.vector.tensor_tensor(dv, fe, fo, OP.subtract)
                return Dt

            def combine_dve(C, Dt):
                k0 = C * gpb
                t0 = 3 * C * gpb
                cDt = cring.tile([BPC, 8 * bt], bf16, name=f"cD{C}", tag="cD")
                updt = cring.tile([BPC, bt], f32, name=f"u{C}", tag="upd")
                dect = cring.tile([BPC, bt], f32, name=f"dec{C}", tag="dec")
                for r in range(3):
                    if r == 0:
                        cv = _mkap(stslab[:, 16 * k0:16 * k0 + 1],
                                   [(16, gpb), (4, 2), (2, 2), (1, 2)])
                    else:
                        cv = _mkap(cfill_b[C][:, 8 * (r - 1):8 * (r - 1) + 1],
                                   [(16, gpb), (4, 2), (2, 2), (1, 2)])
                    drv = _mkap(Dt[:, 8 * r:8 * r + 1],
                                [(24, gpb), (4, 2), (2, 2), (1, 2)])
                    cdv = _mkap(cDt[:, 8 * r:8 * r + 1],
                                [(24, gpb), (4, 2), (2, 2), (1, 2)])
                    nc.vector.tensor_tensor(cdv, cv, drv, OP.mult)
                cd3 = cDt[:, :].rearrange("p (t j) -> p t j", j=8)
                nc.vector.tensor_reduce(updt[:, :], cd3, mybir.AxisListType.X,
                                        OP.add)
                nc.gpsimd.tensor_scalar(dect[:, :], updt[:, :], 0.0, None,
                                        OP.is_lt)
                nc.sync.dma_start(out_d[:, t0:t0 + bt], dect[:, :])

            done_g = set()
            pending = []   # list of (C, Dt)
            # prologue: g + weights for tau 0, chunked to shorten warmup
            chunks0 = ([(0, 12), (12, 16), (28, 16), (44, gpb - 44)]
                       if gpb > 44 else [(0, gpb)])
            m3cur = m3ring.tile([BPC, 128 * gpb], bf16, name="m3_0", tag="m3")
            p2a0 = p2ring.tile([BPC, 32 * gpb], bf16, name="p2a_0", tag="p2a")
            p2b0 = p2ring.tile([BPC, 32 * gpb], bf16, name="p2b_0", tag="p2b")
            beta_init()
            for (k0c, klenc) in chunks0:
                gen_g(3 * k0c, 3 * klenc, f"g0_{k0c}")
                gen_g(3 * ((nb - 1) * gpb + gpb - k0c - klenc), 3 * klenc,
                      f"g{nb - 1}_{k0c}")
                build_m3(0, nb - 1, 0, m3cur, p2a0, p2b0, k0c, klenc)
            done_g.add(0); done_g.add(nb - 1)
            for tau in range(nb):
                A, Bb = tau, nb - 1 - tau
                # pool stages of pending combines go first: their inputs are
                # complete, so Pool chews them while DVE runs this tau's chain
                pend2 = [(C, combine_pool(C)) for C, _ in pending]
                # build NEXT tau's weights now (m3 double-buffered), so the
                # tau+1 chain never waits on Pool
                if tau + 1 < nb:
                    A2, B2 = tau + 1, nb - 2 - tau
                    subeng = None
                    for blk in sorted({A2, B2}):
                        if blk not in done_g:
                            gen_g(3 * blk * gpb, bt, f"g{blk}", eng=subeng)
                            done_g.add(blk)
                    m3n = m3ring.tile([BPC, 128 * gpb], bf16,
                                      name=f"m3_{tau + 1}", tag="m3")
                    p2a = p2ring.tile([BPC, 32 * gpb], bf16,
                                      name=f"p2a_{tau + 1}", tag="p2a")
                    p2b = p2ring.tile([BPC, 32 * gpb], bf16,
                                      name=f"p2b_{tau + 1}", tag="p2b")
                    build_m3(A2, B2, tau + 1, m3n, p2a, p2b, 0, gpb)
                chain_block(tau, m3cur, 0, gpb)
                m3cur = m3n if tau + 1 < nb else None
                fa = cring.tile([BPC, 32 * gpb], bf16, name=f"fa{tau}",
                                tag="fillscratch")
                fills_alpha(A, fa)
                fb = cring.tile([BPC, 32 * gpb], bf16, name=f"fb{tau}",
                                tag="fillscratch")
                fills_beta(Bb, fb)
                for C, Dt in pend2:
                    combine_dve(C, Dt)
                if 2 * tau >= nb - 1:
                    pending = ([(tau, None)] if tau == Bb
                               else [(Bb, None), (A, None)])
                else:
                    pending = []
            ds = [(C, combine_pool(C)) for C, _ in pending]
            for C, Dt in ds:
                combine_dve(C, Dt)
    ctx.__exit__(None, None, None)
    return nc


def _legalize_multiwait(bir):
    """Split multi-sem-wait engine instructions (walrus allows only one)."""
    n = 0
    for fn in bir["functions"]:
        for blk in fn["blocks"]:
            newl = []
            for inst in blk["instructions"]:
                si = inst.get("sync_info") or {}
                waits = si.get("on_wait") or []
                eng = inst.get("engine")
                if len(waits) >= 2 and eng in (
                    "DVE", "Pool", "Activation", "PE", "SP",
                ):
                    for j, w in enumerate(waits):
                        carrier = {
                            "name": inst["name"] + f"-wc{j}",
                            "opcode": "EventSemaphore",
                            "engine": eng,
                            "ins": [],
                            "outs": [],
                            "sync_info": {"on_wait": [w], "on_update": []},
                        }
                        if "debug" in inst:
                            carrier["debug"] = inst["debug"]
                        newl.append(carrier)
                        n += 1
                    si["on_wait"] = []
                    inst["sync_info"] = si
                newl.append(inst)
            blk["instructions"] = newl
    return n


def _finalize(nc):
    import json as _json

    bir = _json.loads(nc.to_json_bytes())
    _legalize_multiwait(bir)
    bts = _json.dumps(bir).encode()
    nc.to_json_bytes = lambda: bts
    return nc


def _prep_inputs(y, h, snr, Tn):
    """Host-side packing: yin rows [y | sp(PIG-ordered)] per word."""
    sigma = np.float32(10.0 ** (-float(snr) / 10.0))
    bits = (np.arange(S)[:, None] >> np.arange(MEM - 1, -1, -1)) & 1
    syms = (1 - 2 * bits).astype(np.float32)
    sp = (syms @ h[:, ::-1].T).astype(np.float32)        # [S, V]
    scale = np.float32(-1.0 / (2.0 * sigma * sigma))
    bias = np.float32(-math.log(math.sqrt(2.0 * math.pi) * sigma))
    Bn = y.shape[0]
    sp_full = sp.T[np.arange(Bn) % V]                    # [Bn, S]
    sp_pig = np.empty_like(sp_full)
    for s in range(S):
        sp_pig[:, PIG[s]] = sp_full[:, s]
    return sp_pig, scale, bias


def kernel(y, h, snr):
    import concourse.bass as bass
    from concourse.bass_utils import run_bass_kernel_spmd

    y = np.ascontiguousarray(np.asarray(y, dtype=np.float32))
    h = np.ascontiguousarray(np.asarray(h, dtype=np.float32))
    snr_f = float(np.asarray(snr))
    tc = 3 * NB * GPB
    Tn = tc + 2
    sp_pig, scale, bias = _prep_inputs(y, h, snr_f, Tn)

    nc = bass.Bass()
    _build(nc, dict(NB=NB, GPB=GPB, NORM_G=NORM_G, scale=scale, bias=bias))
    _finalize(nc)

    in_maps = []
    for c in range(NCORES):
        rows = slice(c * BPC, (c + 1) * BPC)
        yin = np.concatenate([y[rows, :Tn], sp_pig[rows]], axis=1)
        in_maps.append({"yin": np.ascontiguousarray(yin)})
    res = run_bass_kernel_spmd(nc, in_maps, core_ids=list(range(NCORES)))
    dec = np.concatenate([r["dec"] for r in res.results], axis=0)  # [B, tc]

    out = np.zeros((B, T), np.float32)
    out[:, MEM - 1:] = dec[:, :T - (MEM - 1)]
    return out
